# revision 1
# baseline (speedup 1.0000x reference)
"""Mixture-of-Experts (top-2 of 8 experts, erf-GELU FFN) on 8 Trainium2
NeuronCores.

Sharding: experts are grouped (NGRP groups of NSLOT experts each) and the
ffn dim F is split NSPLIT ways within a group, so core = (group, F-slice).
Each core processes the padded token union of its group's experts against
its F-slice of those experts' W1/W2, producing a partial y (summed over
F-slices on the host).  Grouping experts lets the per-group token capacity
average out the per-expert routing imbalance: capacity is sum_i max over
groups of the rank-i expert count, ~= T*top_k/NGRP instead of max_e n_e.

Host side (inside kernel()): router softmax + top-2 + renormalized combine
weights, token dispatch (gather per expert slot) and combine (scatter-add
of the F-slice partial sums).  b2 is applied on the host (it contributes
combine[t,e]*b2[e], independent of the device matmuls).

Device side (Bass/Tile SPMD), per core with F-slice range Foff..Foff+FS:
  MM1: h[fb] = gelu(sum_db w1[db,fb]^T @ xT[db] + b1[fb])   (tokens moving)
  MM2: y[dt] = (sum_fb w2[fb,dt]^T @ h[fb]) * wg            (tokens moving)
bf16 matmuls, fp32 PSUM accumulation, bf16 partial-y output.

Layouts shipped per core (P=128 partitions, C = padded group capacity):
  x   [P, sum_k NB_D*cn_k] bf16  chunk-blocked x^T: block k holds
                                 [p, db, c] = x[off_k+c, db*128+p]
  w1  [NB_F, P, NSLOT, NB_D, P]  [fb,p,s,db,m] = W1[e_s][db*128+p, Foff+fb*128+m]
  w2  [NB_D, P, NSLOT, NB_F, P]  [dt,k,s,fb,m] = W2[e_s][Foff+fb*128+k, dt*128+m]
  b1  [P, NSLOT*NB_F] f32        [p, s*NB_F+fb] = b1[e_s][Foff+fb*128+p]
  wg  [P, C] f32                 combine weight per token slot, bcast over p
  out [P, NB_D, C] bf16          partial y^T * wg

DMA: one hand-ordered stream on the sync queue (x chunks interleaved with
w1 slabs so MM1 can start early; w1 stays fully resident; w2 dt-slabs
stream just-in-time during MM2 from a ring pool).  Outputs go on the
scalar queue; small constants on gpsimd.

The kernel opens with ~24 dummy matmuls on a memset scratch tile: the PE's
HAM clock gate holds the array at 1.2 GHz until it has seen ~3.4 us of
sustained activity, so warming it during the dead DMA-head window lets the
real matmuls run at the full 2.4 GHz from the first instruction.
"""

import itertools

import numpy as np
import ml_dtypes

P = 128
N_CORES = 8
D, F, E = 1024, 4096, 8
NB_D = D // P
TOP_K = 2

NGRP = 4                 # expert groups (pairs); cores per group = 8/NGRP
NSLOT = E // NGRP        # experts per group
NSPLIT = N_CORES // NGRP # F-split ways within a group
FS = F // NSPLIT
NB_F = FS // P

MAX_CHUNK = 512          # PSUM bank = 512 fp32 columns

_cache = {}
_last_in_maps = None


HEAD_CHUNK = 128  # small first chunk so MM1 starts during the DMA ramp


def _chunk_plan(S):
    """Per-slot even chunking: list of (slot, off, cn, xoff), C, XW.

    Slot 0 leads with a small chunk: its x block + first w1 tile are the
    critical DMA prefix before the first matmul can issue.
    """
    chunks = []
    off = 0
    xoff = 0
    for s, sz in enumerate(S):
        sizes = []
        rem = sz
        if s == 0 and HEAD_CHUNK + P < rem <= HEAD_CHUNK + MAX_CHUNK:
            # lead with a small chunk without increasing the chunk count
            sizes.append(HEAD_CHUNK)
            rem -= HEAD_CHUNK
        n_ch = max(1, -(-rem // MAX_CHUNK))
        c0 = 0
        for i in range(n_ch):
            cn = (rem - c0 + (n_ch - 1 - i)) // (n_ch - i)
            sizes.append(cn)
            c0 += cn
        c0 = 0
        for cn in sizes:
            chunks.append((s, off + c0, cn, xoff))
            xoff += NB_D * cn
            c0 += cn
        off += sz
    return chunks, off, xoff


def _chunk_plan2(S):
    """MM2's chunk grid: per-slot even chunks, no head split.  MM2 reads h
    and writes out by column range, so its grid needn't match MM1's."""
    chunks = []
    off = 0
    for s, sz in enumerate(S):
        n_ch = max(1, -(-sz // MAX_CHUNK))
        c0 = 0
        for i in range(n_ch):
            cn = (sz - c0 + (n_ch - 1 - i)) // (n_ch - i)
            chunks.append((s, off + c0, cn))
            c0 += cn
        off += sz
    return chunks


def _plan(counts):
    """Group experts into NGRP groups of NSLOT, slot-ordered by count desc.

    Returns (groups, S): groups[g] = expert ids in slot order, S[i] = padded
    slot-i capacity = max over groups of the slot-i expert count.
    """
    counts = np.asarray(counts)
    order = np.argsort(-counts, kind="stable")
    if NSLOT == 1:
        groups = [[int(e)] for e in order]
        S = [int(counts[order[0]])]
    elif NGRP == 4:
        # pairs: rank i with rank i+4 minimizes n_max + max-of-partners
        groups = [[int(order[i]), int(order[i + 4])] for i in range(4)]
        S = [int(counts[order[0]]), int(counts[order[4]])]
    elif NGRP == 2:
        best = None
        for g0 in itertools.combinations(range(E), NSLOT):
            if 0 not in g0:
                continue
            g1 = tuple(e for e in range(E) if e not in g0)
            a = sorted((int(counts[e]) for e in g0), reverse=True)
            b = sorted((int(counts[e]) for e in g1), reverse=True)
            S_ = [max(a[i], b[i]) for i in range(NSLOT)]
            if best is None or sum(S_) < sum(best[0]):
                o0 = sorted(g0, key=lambda e: -counts[e])
                o1 = sorted(g1, key=lambda e: -counts[e])
                best = (S_, [list(o0), list(o1)])
        S, groups = best
    else:
        raise ValueError(NGRP)
    return groups, [max(s, 1) for s in S]


def _build(S):
    """Build + compile the per-core SPMD Bass program for slot sizes S."""
    from concourse import bacc
    import concourse.tile as tile
    import concourse.mybir as mybir

    chunks, C, XW = _chunk_plan(S)
    chunks2 = _chunk_plan2(S)
    bf16 = mybir.dt.bfloat16
    f32 = mybir.dt.float32
    GELU = mybir.ActivationFunctionType.Gelu

    nc = bacc.Bacc(None, target_bir_lowering=False)
    x_d = nc.dram_tensor("x", [P, XW], bf16, kind="ExternalInput")
    w1_d = nc.dram_tensor("w1", [NB_F, P, NSLOT, NB_D, P], bf16, kind="ExternalInput")
    w2_d = nc.dram_tensor("w2", [NB_D, P, NSLOT, NB_F, P], bf16, kind="ExternalInput")
    b1_d = nc.dram_tensor("b1", [P, NSLOT * NB_F], f32, kind="ExternalInput")
    wg_d = nc.dram_tensor("wg", [P, C], f32, kind="ExternalInput")
    out_d = nc.dram_tensor("out", [P, NB_D, C], bf16, kind="ExternalOutput")

    with tile.TileContext(nc) as tc:
        with (
            tc.tile_pool(name="const", bufs=1) as const,
            tc.tile_pool(name="xp", bufs=1) as xp,
            tc.tile_pool(name="w1p", bufs=1) as w1p,
            tc.tile_pool(name="w2p", bufs=4) as w2p,
            tc.tile_pool(name="ps1", bufs=5, space="PSUM") as ps1p,
            tc.tile_pool(name="ps2", bufs=3, space="PSUM") as ps2p,
            tc.tile_pool(name="outp", bufs=2) as outp,
        ):
            b1_t = const.tile([P, NSLOT * NB_F], f32)
            wg_t = const.tile([P, C], f32)
            h_t = const.tile([P, NB_F, C], bf16)
            scr_t = const.tile([P, MAX_CHUNK], bf16)

            nc.gpsimd.dma_start(b1_t[:], b1_d[:])

            # PE warm-up: the HAM clock gate runs the PE at 1.2 GHz until
            # it has seen ~3.4 us of sustained activity (measured: the
            # clock reaches 8/8 only ~10 us into MM1 otherwise).  Burn the
            # dead head window (preamble + first DMA latency) on dummy
            # matmuls over uninitialized scratch so the real matmuls start
            # at full clock.  Results land in a PSUM bank MM2 later resets.
            nc.vector.memset(scr_t[:], 0.0)
            wps = ps2p.tile([P, MAX_CHUNK], f32, name="ps2", tag="ps2")
            NWARM = 24
            for i in range(NWARM):
                nc.tensor.matmul(
                    wps[:],
                    lhsT=scr_t[:, :P],
                    rhs=scr_t[:],
                    start=(i == 0),
                    stop=(i == NWARM - 1),
                )

            # All critical input DMA rides the sync queue (hardware DGE)
            # in priority order: x chunk 0, then (fb0, slot0)'s w1 tile —
            # the first matmul's exact needs — then x chunk 1, the rest of
            # fb0's w1, the remaining x, and the remaining w1 slabs.  One
            # queue = strict FIFO = the hand-ordered stream; splitting
            # across queues just splits the shared SDMA engines (measured
            # slower).  Sub-transfers at the head pipeline the ring deeper
            # during its ramp, and subtile deps unblock each db's matmuls
            # as its slice lands.  w2 streams during MM2 (ring pool).
            # single resident tiles for x and w1 (fewer tiles = fewer
            # semaphores and release instructions); per-slice DMAs retain
            # fine-grained availability via subtile deps
            x_t = xp.tile([P, XW], bf16, name="x_t", tag="x_t")
            w1_t = w1p.tile(
                [P, NB_F, NSLOT, NB_D, P], bf16, name="w1_t", tag="w1_t"
            )

            def _x_dma(k, eng, n_sub=2):
                s, off, cn, xoff = chunks[k]
                step = NB_D // n_sub * cn
                for i in range(n_sub):
                    eng.dma_start(
                        x_t[:, xoff + i * step : xoff + (i + 1) * step],
                        x_d[:, xoff + i * step : xoff + (i + 1) * step],
                    )

            _x_dma(0, nc.sync)
            nc.sync.dma_start(w1_t[:, 0, 0], w1_d[0][:, 0])
            _x_dma(1, nc.sync)
            for s in range(1, NSLOT):
                nc.sync.dma_start(w1_t[:, 0, s], w1_d[0][:, s])
            for k in range(2, len(chunks)):
                _x_dma(k, nc.sync)
            for k in range(1, NB_F):
                for s in range(NSLOT):
                    nc.sync.dma_start(w1_t[:, k, s], w1_d[k][:, s])

            nc.gpsimd.dma_start(wg_t[:], wg_d[:])

            # ---- MM1: h[fb] = gelu(sum_db w1^T @ x + b1), tokens moving
            for fb in range(NB_F):
                for k, (s, off, cn, xoff) in enumerate(chunks):
                    ps = ps1p.tile([P, MAX_CHUNK], f32)
                    for db in range(NB_D):
                        nc.tensor.matmul(
                            ps[:, :cn],
                            lhsT=w1_t[:, fb, s, db, :],
                            rhs=x_t[:, xoff + db * cn : xoff + (db + 1) * cn],
                            start=(db == 0),
                            stop=(db == NB_D - 1),
                        )
                    nc.scalar.activation(
                        h_t[:, fb, off : off + cn],
                        ps[:, :cn],
                        GELU,
                        bias=b1_t[:, s * NB_F + fb : s * NB_F + fb + 1],
                    )

            # ---- MM2: y[dt] = (sum_fb w2^T @ h) * wg, tokens moving.
            # w2 dt-slabs stream just-in-time from a ring pool; the sync
            # queue is empty of w1/x by now so slabs run ~3 dts ahead.
            for dt in range(NB_D):
                w2_t = w2p.tile([P, NSLOT, NB_F, P], bf16, name="w2d", tag="w2d")
                nc.sync.dma_start(w2_t[:], w2_d[dt])
                o_t = outp.tile([P, C], bf16, name="o", tag="o")
                dt_chunks = chunks2
                if dt == NB_D - 1:
                    # end on the smallest chunk: its eviction + out DMA +
                    # HBM write receipt are the kernel's exposed tail
                    dt_chunks = sorted(chunks2, key=lambda c: -c[2])
                for s, off, cn in dt_chunks:
                    ps = ps2p.tile([P, MAX_CHUNK], f32, name="ps2", tag="ps2")
                    for fb in range(NB_F):
                        nc.tensor.matmul(
                            ps[:, :cn],
                            lhsT=w2_t[:, s, fb, :],
                            rhs=h_t[:, fb, off : off + cn],
                            start=(fb == 0),
                            stop=(fb == NB_F - 1),
                        )
                    nc.vector.scalar_tensor_tensor(
                        o_t[:, off : off + cn],
                        ps[:, :cn],
                        0.0,
                        wg_t[:, off : off + cn],
                        op0=mybir.AluOpType.add,
                        op1=mybir.AluOpType.mult,
                    )
                    # per-chunk output DMA overlaps the remaining chunks'
                    # matmuls, so the kernel tail is one chunk, not one dt;
                    # scalar queue — idle once x has landed
                    nc.scalar.dma_start(
                        out_d[:, dt, off : off + cn], o_t[:, off : off + cn]
                    )

    nc.compile()
    return nc


def _route(x, W_router):
    """Top-2 routing, replicating jax softmax/top_k/renorm semantics."""
    T = x.shape[0]
    logits = x @ np.asarray(W_router, np.float32)
    m = logits.max(axis=1, keepdims=True)
    ex = np.exp(logits - m, dtype=np.float32)
    probs = ex / ex.sum(axis=1, keepdims=True, dtype=np.float32)
    r = np.arange(T)
    i1 = probs.argmax(axis=1)
    masked = probs.copy()
    masked[r, i1] = -np.inf
    i2 = masked.argmax(axis=1)
    p1 = probs[r, i1]
    p2 = probs[r, i2]
    s = p1 + p2
    return i1, i2, p1 / s, p2 / s


def kernel(hidden_states, W_router, W1, b1, W2, b2):
    from concourse.bass_utils import run_bass_kernel_spmd

    B, S_, D_ = hidden_states.shape
    T = B * S_
    x = np.ascontiguousarray(np.asarray(hidden_states, np.float32).reshape(T, D_))

    i1, i2, w1c, w2c = _route(x, W_router)

    idxs, wgts = [], []
    for e in range(E):
        sel1 = i1 == e
        sel2 = i2 == e
        idx = np.nonzero(sel1 | sel2)[0]
        w = np.where(sel1[idx], w1c[idx], w2c[idx]).astype(np.float32)
        idxs.append(idx)
        wgts.append(w)

    counts = [len(ix) for ix in idxs]
    groups, S = _plan(counts)
    chunks, C, XW = _chunk_plan(S)
    offs = np.concatenate([[0], np.cumsum(S)])[:NSLOT]

    key = tuple(S)
    if key not in _cache:
        _cache[key] = _build(S)
    nc = _cache[key]

    bf16 = ml_dtypes.bfloat16
    xb = x.astype(bf16)
    W1f = np.asarray(W1, np.float32)
    W2f = np.asarray(W2, np.float32)
    b1f = np.asarray(b1, np.float32)

    in_maps = [None] * N_CORES
    for g, grp in enumerate(groups):
        # group-shared: x (chunk-blocked x^T) and wg
        xg = np.zeros((C, D), bf16)
        wfull = np.zeros(C, np.float32)
        for s, e in enumerate(grp):
            n = counts[e]
            xg[offs[s] : offs[s] + n] = xb[idxs[e]]
            wfull[offs[s] : offs[s] + n] = wgts[e]
        x_arr = np.empty((P, XW), bf16)
        for s, off, cn, xoff in chunks:
            x_arr[:, xoff : xoff + NB_D * cn] = (
                xg[off : off + cn]
                .T.reshape(NB_D, P, cn)
                .transpose(1, 0, 2)
                .reshape(P, NB_D * cn)
            )
        wgb = np.ascontiguousarray(np.broadcast_to(wfull, (P, C)))

        for j in range(NSPLIT):
            foff = j * FS
            w1e = np.ascontiguousarray(
                W1f[grp][:, :, foff : foff + FS]
                .astype(bf16)
                .reshape(NSLOT, NB_D, P, NB_F, P)
                .transpose(3, 2, 0, 1, 4)
            )
            w2e = np.ascontiguousarray(
                W2f[grp][:, foff : foff + FS, :]
                .astype(bf16)
                .reshape(NSLOT, NB_F, P, NB_D, P)
                .transpose(3, 2, 0, 1, 4)
            )
            b1e = np.ascontiguousarray(
                b1f[grp][:, foff : foff + FS]
                .reshape(NSLOT, NB_F, P)
                .transpose(2, 0, 1)
                .reshape(P, NSLOT * NB_F)
            )
            in_maps[g * NSPLIT + j] = {
                "x": x_arr,
                "w1": w1e,
                "w2": w2e,
                "b1": b1e,
                "wg": wgb,
            }

    global _last_in_maps
    _last_in_maps = in_maps

    res = run_bass_kernel_spmd(nc, in_maps, core_ids=list(range(N_CORES)))

    out = np.zeros((T, D), np.float32)
    b2f = np.asarray(b2, np.float32)
    for g, grp in enumerate(groups):
        acc = np.zeros((P, NB_D, C), np.float32)
        for j in range(NSPLIT):
            acc += np.asarray(res.results[g * NSPLIT + j]["out"], np.float32)
        for s, e in enumerate(grp):
            n = counts[e]
            y = acc[:, :, offs[s] : offs[s] + n].transpose(2, 1, 0).reshape(n, D)
            out[idxs[e]] += y
            if b2f[e].any():
                out[idxs[e]] += wgts[e][:, None] * b2f[e][None, :]
    return out.reshape(B, S_, D_).astype(np.float32)

